# revision 7
# baseline (speedup 1.0000x reference)
"""Haar DWT-1D forward on 8 Trainium2 NeuronCores (Bass/Tile).

reference:  lfc = einsum('ncl,kl->nck', x, matrix_low)
            hfc = einsum('ncl,kl->nck', x, matrix_high)
with matrix_low/matrix_high the structured 2-tap haar analysis matrices:
row k of matrix_low  holds [a, b] at columns (2k, 2k+1)  (a = b = 1/sqrt2)
row k of matrix_high holds [c, d] at columns (2k, 2k+1)  (c = -1/sqrt2, d = 1/sqrt2)

So per (n, c) row:  lfc[k] = a*x[2k] + b*x[2k+1]
                    hfc[k] = c*x[2k] + d*x[2k+1]
i.e. a pure memory-bound strided 2-tap filter — no matmul needed.

Sharding: data-parallel along N (32 -> 4 per core, no cross-core comm).
Each core processes a (256, 8192) row-block; using a == b and c == -d:
  lfc = (even + odd) * a   (VectorE tensor_add, ScalarE activation-mul)
  hfc = (odd - even) * d   (VectorE tensor_sub, ScalarE activation-mul)
(The fused scalar_tensor_tensor op would halve the instruction count, but
its ISA struct overflows on the sync-wait commands Tile attaches to it —
neuronx-cc "Too many sync wait commands" — so TT + ACT-mul it is.)
"""

import numpy as np

_N, _C, _L1 = 32, 64, 8192
_L = _L1 // 2
_NCORES = 8
_NS = _N // _NCORES          # batch rows per core (4)
_ROWS = _NS * _C             # sbuf-partition rows per core (256)
_P = 128                     # partitions per tile
_FCH = 4096                  # input free-dim chunk per tile (16 KiB/partition)

_cache = {}


def _build_program(a, b, c, d):
    """Emit the per-core Bass program. All 8 cores run this same program
    on their own (256, 8192) shard."""
    import concourse.tile as tile
    from concourse import bacc, mybir

    # Bacc (not raw Bass): its compile pipeline runs generate_event_semaphores,
    # which splits multi-wait instructions — TRN2 allows only 1 sync wait per
    # instruction and neuronx-cc hard-errors otherwise. target_bir_lowering
    # must be off so walrus gets pre-lowered IR (the run_kernel test path).
    nc = bacc.Bacc("TRN2", target_bir_lowering=False, debug=False,
                   num_devices=_NCORES)
    x = nc.dram_tensor("x", [_ROWS, _L1], mybir.dt.float32, kind="ExternalInput")
    lo = nc.dram_tensor("lo", [_ROWS, _L], mybir.dt.float32, kind="ExternalOutput")
    hi = nc.dram_tensor("hi", [_ROWS, _L], mybir.dt.float32, kind="ExternalOutput")

    # Fast path needs a == b (lfc = (even+odd)*a) and c == -d
    # (hfc = (odd-even)*d). True for any orthogonal haar-style 2-tap pair.
    fast = (abs(a - b) <= 1e-12 * (abs(a) + abs(b))
            and abs(c + d) <= 1e-12 * (abs(c) + abs(d)))

    with tile.TileContext(nc) as tc:
        with tc.tile_pool(name="io", bufs=3) as pool:
            for r in range(0, _ROWS, _P):
                for f in range(0, _L1, _FCH):
                    kw = _FCH // 2
                    k0 = f // 2  # output col start for this chunk
                    t = pool.tile([_P, _FCH], mybir.dt.float32, tag="in")
                    nc.sync.dma_start(out=t[:], in_=x[r:r + _P, f:f + _FCH])
                    even = t[:, 0:_FCH:2]
                    odd = t[:, 1:_FCH:2]

                    lo_t = pool.tile([_P, kw], mybir.dt.float32, tag="lo")
                    hi_t = pool.tile([_P, kw], mybir.dt.float32, tag="hi")
                    if fast:
                        s = pool.tile([_P, kw], mybir.dt.float32, tag="s")
                        nc.vector.tensor_add(s[:], even, odd)
                        nc.scalar.mul(lo_t[:], s[:], float(a))
                        g = pool.tile([_P, kw], mybir.dt.float32, tag="g")
                        nc.vector.tensor_sub(g[:], odd, even)
                        nc.scalar.mul(hi_t[:], g[:], float(d))
                    else:
                        u = pool.tile([_P, kw], mybir.dt.float32, tag="u")
                        w = pool.tile([_P, kw], mybir.dt.float32, tag="w")
                        nc.scalar.mul(u[:], even, float(a))
                        nc.vector.tensor_scalar_mul(w[:], odd, float(b))
                        nc.vector.tensor_add(lo_t[:], u[:], w[:])
                        nc.scalar.mul(u[:], even, float(c))
                        nc.vector.tensor_scalar_mul(w[:], odd, float(d))
                        nc.vector.tensor_add(hi_t[:], u[:], w[:])

                    nc.sync.dma_start(out=lo[r:r + _P, k0:k0 + kw], in_=lo_t[:])
                    nc.sync.dma_start(out=hi[r:r + _P, k0:k0 + kw], in_=hi_t[:])
    nc.finalize()  # runs the Bacc compile pipeline (reg alloc, wait splitting)
    return nc


def kernel(input, matrix_low, matrix_high, _trace=False):
    from concourse.bass_utils import run_bass_kernel_spmd

    x = np.ascontiguousarray(np.asarray(input, dtype=np.float32))
    ml = np.asarray(matrix_low, dtype=np.float32)
    mh = np.asarray(matrix_high, dtype=np.float32)
    assert x.shape == (_N, _C, _L1), x.shape

    # The transform matrices are structured 2-tap banded: row k carries its
    # two taps at columns (2k, 2k+1), identical for every k. Extract them.
    a, b = float(ml[0, 0]), float(ml[0, 1])
    c, d = float(mh[0, 0]), float(mh[0, 1])

    key = (a, b, c, d)
    if key not in _cache:
        _cache[key] = _build_program(a, b, c, d)
    nc = _cache[key]

    in_maps = [
        {"x": x[i * _NS:(i + 1) * _NS].reshape(_ROWS, _L1)}
        for i in range(_NCORES)
    ]
    res = run_bass_kernel_spmd(
        nc, in_maps, core_ids=list(range(_NCORES)), trace=_trace)
    kernel.last_run = res

    lfc = np.concatenate(
        [res.results[i]["lo"].reshape(_NS, _C, _L) for i in range(_NCORES)], axis=0)
    hfc = np.concatenate(
        [res.results[i]["hi"].reshape(_NS, _C, _L) for i in range(_NCORES)], axis=0)
    return lfc, hfc


# revision 10
# speedup vs baseline: 1.1454x; 1.1454x over previous
"""Haar DWT-1D forward on 8 Trainium2 NeuronCores (Bass/Tile).

reference:  lfc = einsum('ncl,kl->nck', x, matrix_low)
            hfc = einsum('ncl,kl->nck', x, matrix_high)
with matrix_low/matrix_high the structured 2-tap haar analysis matrices:
row k of matrix_low  holds [a, b] at columns (2k, 2k+1)  (a = b = 1/sqrt2)
row k of matrix_high holds [c, d] at columns (2k, 2k+1)  (c = -1/sqrt2, d = 1/sqrt2)

So per (n, c) row:  lfc[k] = a*x[2k] + b*x[2k+1]
                    hfc[k] = c*x[2k] + d*x[2k+1]
i.e. a pure memory-bound strided 2-tap filter — no matmul needed.

Sharding: data-parallel along N (32 -> 4 per core, no cross-core comm).
Each core processes a (256, 8192) row-block; using a == b and c == -d:
  lfc = (even + odd) * a   (VectorE tensor_add, ScalarE activation-mul)
  hfc = (odd - even) * d   (VectorE tensor_sub, ScalarE activation-mul)
(The fused scalar_tensor_tensor op would halve the instruction count, but
its ISA struct overflows on the sync-wait commands Tile attaches to it —
neuronx-cc "Too many sync wait commands" — so TT + ACT-mul it is.)
"""

import numpy as np

_N, _C, _L1 = 32, 64, 8192
_L = _L1 // 2
_NCORES = 8
_NS = _N // _NCORES          # batch rows per core (4)
_ROWS = _NS * _C             # sbuf-partition rows per core (256)
_P = 128                     # partitions per tile
_FCH = 2048                  # input free-dim chunk per tile (8 KiB/partition)

_cache = {}


def _build_program(a, b, c, d):
    """Emit the per-core Bass program. All 8 cores run this same program
    on their own (256, 8192) shard."""
    import concourse.tile as tile
    from concourse import bacc, mybir

    # Bacc (not raw Bass): its compile pipeline runs generate_event_semaphores,
    # which splits multi-wait instructions — TRN2 allows only 1 sync wait per
    # instruction and neuronx-cc hard-errors otherwise. target_bir_lowering
    # must be off so walrus gets pre-lowered IR (the run_kernel test path).
    nc = bacc.Bacc("TRN2", target_bir_lowering=False, debug=False,
                   num_devices=_NCORES)
    x = nc.dram_tensor("x", [_ROWS, _L1], mybir.dt.float32, kind="ExternalInput")
    lo = nc.dram_tensor("lo", [_ROWS, _L], mybir.dt.float32, kind="ExternalOutput")
    hi = nc.dram_tensor("hi", [_ROWS, _L], mybir.dt.float32, kind="ExternalOutput")

    # Fast path needs a == b (lfc = (even+odd)*a) and c == -d
    # (hfc = (odd-even)*d). True for any orthogonal haar-style 2-tap pair.
    fast = (abs(a - b) <= 1e-12 * (abs(a) + abs(b))
            and abs(c + d) <= 1e-12 * (abs(c) + abs(d)))

    with tile.TileContext(nc) as tc:
        with tc.tile_pool(name="io", bufs=4) as pool:
            for r in range(0, _ROWS, _P):
                for f in range(0, _L1, _FCH):
                    kw = _FCH // 2
                    k0 = f // 2  # output col start for this chunk
                    t = pool.tile([_P, _FCH], mybir.dt.float32, tag="in")
                    nc.sync.dma_start(out=t[:], in_=x[r:r + _P, f:f + _FCH])
                    even = t[:, 0:_FCH:2]
                    odd = t[:, 1:_FCH:2]

                    lo_t = pool.tile([_P, kw], mybir.dt.float32, tag="lo")
                    hi_t = pool.tile([_P, kw], mybir.dt.float32, tag="hi")
                    if fast:
                        s = pool.tile([_P, kw], mybir.dt.float32, tag="s")
                        nc.vector.tensor_add(s[:], even, odd)
                        nc.scalar.mul(lo_t[:], s[:], float(a))
                        g = pool.tile([_P, kw], mybir.dt.float32, tag="g")
                        nc.vector.tensor_sub(g[:], odd, even)
                        nc.scalar.mul(hi_t[:], g[:], float(d))
                    else:
                        u = pool.tile([_P, kw], mybir.dt.float32, tag="u")
                        w = pool.tile([_P, kw], mybir.dt.float32, tag="w")
                        nc.scalar.mul(u[:], even, float(a))
                        nc.vector.tensor_scalar_mul(w[:], odd, float(b))
                        nc.vector.tensor_add(lo_t[:], u[:], w[:])
                        nc.scalar.mul(u[:], even, float(c))
                        nc.vector.tensor_scalar_mul(w[:], odd, float(d))
                        nc.vector.tensor_add(hi_t[:], u[:], w[:])

                    # outputs on the ACT HWDGE ring (qActDynamicHW), inputs
                    # on the SP ring — two queues interleave at packet
                    # granularity instead of FIFO-blocking on one ring
                    nc.scalar.dma_start(out=lo[r:r + _P, k0:k0 + kw], in_=lo_t[:])
                    nc.scalar.dma_start(out=hi[r:r + _P, k0:k0 + kw], in_=hi_t[:])
    nc.finalize()  # runs the Bacc compile pipeline (reg alloc, wait splitting)
    return nc


def kernel(input, matrix_low, matrix_high, _trace=False):
    from concourse.bass_utils import run_bass_kernel_spmd

    x = np.ascontiguousarray(np.asarray(input, dtype=np.float32))
    ml = np.asarray(matrix_low, dtype=np.float32)
    mh = np.asarray(matrix_high, dtype=np.float32)
    assert x.shape == (_N, _C, _L1), x.shape

    # The transform matrices are structured 2-tap banded: row k carries its
    # two taps at columns (2k, 2k+1), identical for every k. Extract them.
    a, b = float(ml[0, 0]), float(ml[0, 1])
    c, d = float(mh[0, 0]), float(mh[0, 1])

    key = (a, b, c, d)
    if key not in _cache:
        _cache[key] = _build_program(a, b, c, d)
    nc = _cache[key]

    in_maps = [
        {"x": x[i * _NS:(i + 1) * _NS].reshape(_ROWS, _L1)}
        for i in range(_NCORES)
    ]
    res = run_bass_kernel_spmd(
        nc, in_maps, core_ids=list(range(_NCORES)), trace=_trace)
    kernel.last_run = res

    lfc = np.concatenate(
        [res.results[i]["lo"].reshape(_NS, _C, _L) for i in range(_NCORES)], axis=0)
    hfc = np.concatenate(
        [res.results[i]["hi"].reshape(_NS, _C, _L) for i in range(_NCORES)], axis=0)
    return lfc, hfc


# revision 15
# speedup vs baseline: 1.1510x; 1.0049x over previous
"""Haar DWT-1D forward on 8 Trainium2 NeuronCores (Bass/Tile).

reference:  lfc = einsum('ncl,kl->nck', x, matrix_low)
            hfc = einsum('ncl,kl->nck', x, matrix_high)
with matrix_low/matrix_high the structured 2-tap haar analysis matrices:
row k of matrix_low  holds [a, b] at columns (2k, 2k+1)  (a = b = 1/sqrt2)
row k of matrix_high holds [c, d] at columns (2k, 2k+1)  (c = -1/sqrt2, d = 1/sqrt2)

So per (n, c) row:  lfc[k] = a*x[2k] + b*x[2k+1]
                    hfc[k] = c*x[2k] + d*x[2k+1]
i.e. a pure memory-bound strided 2-tap filter — no matmul needed.

Sharding: data-parallel along N (32 -> 4 per core, no cross-core comm).
Each core processes a (256, 8192) row-block; using a == b and c == -d:
  lfc = (even + odd) * a   (VectorE tensor_add, ScalarE activation-mul)
  hfc = (odd - even) * d   (VectorE tensor_sub, ScalarE activation-mul)
(The fused scalar_tensor_tensor op would halve the instruction count, but
its ISA struct overflows on the sync-wait commands Tile attaches to it —
neuronx-cc "Too many sync wait commands" — so TT + ACT-mul it is.)
"""

import numpy as np

_N, _C, _L1 = 32, 64, 8192
_L = _L1 // 2
_NCORES = 8
_NS = _N // _NCORES          # batch rows per core (4)
_ROWS = _NS * _C             # sbuf-partition rows per core (256)
_P = 128                     # partitions per tile
_FCH = 2048                  # input free-dim chunk per tile (8 KiB/partition)

_cache = {}


def _build_program(a, b, c, d):
    """Emit the per-core Bass program. All 8 cores run this same program
    on their own (256, 8192) shard."""
    import concourse.tile as tile
    from concourse import bacc, mybir

    # Bacc (not raw Bass): its compile pipeline runs generate_event_semaphores,
    # which splits multi-wait instructions — TRN2 allows only 1 sync wait per
    # instruction and neuronx-cc hard-errors otherwise. target_bir_lowering
    # must be off so walrus gets pre-lowered IR (the run_kernel test path).
    nc = bacc.Bacc("TRN2", target_bir_lowering=False, debug=False,
                   num_devices=_NCORES)
    x = nc.dram_tensor("x", [_ROWS, _L1], mybir.dt.float32, kind="ExternalInput")
    lo = nc.dram_tensor("lo", [_ROWS, _L], mybir.dt.float32, kind="ExternalOutput")
    hi = nc.dram_tensor("hi", [_ROWS, _L], mybir.dt.float32, kind="ExternalOutput")

    # Fast path needs a == b (lfc = (even+odd)*a) and c == -d
    # (hfc = (odd-even)*d). True for any orthogonal haar-style 2-tap pair.
    fast = (abs(a - b) <= 1e-12 * (abs(a) + abs(b))
            and abs(c + d) <= 1e-12 * (abs(c) + abs(d)))

    with tile.TileContext(nc) as tc:
        with tc.tile_pool(name="io", bufs=4) as pool:
            for r in range(0, _ROWS, _P):
                for f in range(0, _L1, _FCH):
                    kw = _FCH // 2
                    k0 = f // 2  # output col start for this chunk
                    t = pool.tile([_P, _FCH], mybir.dt.float32, tag="in")
                    nc.sync.dma_start(out=t[:], in_=x[r:r + _P, f:f + _FCH])
                    even = t[:, 0:_FCH:2]
                    odd = t[:, 1:_FCH:2]

                    lo_t = pool.tile([_P, kw], mybir.dt.float32, tag="lo")
                    hi_t = pool.tile([_P, kw], mybir.dt.float32, tag="hi")
                    if fast:
                        s = pool.tile([_P, kw], mybir.dt.float32, tag="s")
                        nc.vector.tensor_add(s[:], even, odd)
                        nc.scalar.mul(lo_t[:], s[:], float(a))
                        g = pool.tile([_P, kw], mybir.dt.float32, tag="g")
                        nc.vector.tensor_sub(g[:], odd, even)
                        nc.scalar.mul(hi_t[:], g[:], float(d))
                    else:
                        u = pool.tile([_P, kw], mybir.dt.float32, tag="u")
                        w = pool.tile([_P, kw], mybir.dt.float32, tag="w")
                        nc.scalar.mul(u[:], even, float(a))
                        nc.vector.tensor_scalar_mul(w[:], odd, float(b))
                        nc.vector.tensor_add(lo_t[:], u[:], w[:])
                        nc.scalar.mul(u[:], even, float(c))
                        nc.vector.tensor_scalar_mul(w[:], odd, float(d))
                        nc.vector.tensor_add(hi_t[:], u[:], w[:])

                    # split stores across both HWDGE rings (SP + ACT) so the
                    # two queues interleave at packet granularity instead of
                    # FIFO-blocking behind the input loads on one ring
                    nc.scalar.dma_start(out=lo[r:r + _P, k0:k0 + kw], in_=lo_t[:])
                    nc.sync.dma_start(out=hi[r:r + _P, k0:k0 + kw], in_=hi_t[:])
    nc.finalize()  # runs the Bacc compile pipeline (reg alloc, wait splitting)
    return nc


def kernel(input, matrix_low, matrix_high, _trace=False):
    from concourse.bass_utils import run_bass_kernel_spmd

    x = np.ascontiguousarray(np.asarray(input, dtype=np.float32))
    ml = np.asarray(matrix_low, dtype=np.float32)
    mh = np.asarray(matrix_high, dtype=np.float32)
    assert x.shape == (_N, _C, _L1), x.shape

    # The transform matrices are structured 2-tap banded: row k carries its
    # two taps at columns (2k, 2k+1), identical for every k. Extract them.
    a, b = float(ml[0, 0]), float(ml[0, 1])
    c, d = float(mh[0, 0]), float(mh[0, 1])

    key = (a, b, c, d)
    if key not in _cache:
        _cache[key] = _build_program(a, b, c, d)
    nc = _cache[key]

    in_maps = [
        {"x": x[i * _NS:(i + 1) * _NS].reshape(_ROWS, _L1)}
        for i in range(_NCORES)
    ]
    res = run_bass_kernel_spmd(
        nc, in_maps, core_ids=list(range(_NCORES)), trace=_trace)
    kernel.last_run = res

    lfc = np.concatenate(
        [res.results[i]["lo"].reshape(_NS, _C, _L) for i in range(_NCORES)], axis=0)
    hfc = np.concatenate(
        [res.results[i]["hi"].reshape(_NS, _C, _L) for i in range(_NCORES)], axis=0)
    return lfc, hfc
